# revision 26
# baseline (speedup 1.0000x reference)
"""Sliding-window multi-head attention for Trainium2, 8-core SPMD.

Sharding: sequence-parallel. B=2 batches x 4 chunks of 512 queries = 8 cores.
Each core computes QKV projections for its chunk (+128-row halo for K/V),
banded attention (window 256 -> band |j-s|<=128), and the output projection
for its 512 rows. No collectives; host concatenates the 8 output chunks.

Math notes (validated against the reference):
 - The reference's clamped scatter-add with zero-padded keys is exactly a
   banded score matrix: full[s,j] = q_s.k_j / 8 for |j-s|<=128, -inf outside.
 - Softmax computed without max-subtraction (scores are O(1), no overflow).
 - Denominators come free from the AV matmul via a ones-column on V (M=65).
 - Attention is computed transposed (scores^T[key, query]) so no transposes
   are needed anywhere in the hot loop; q^T/k^T come straight out of the
   projection, V is projected in natural layout for the AV lhsT.
"""

import numpy as np

import concourse.bass as bass
import concourse.tile as tile
from concourse import mybir
from concourse.alu_op_type import AluOpType
from concourse.vector_clock import ScopedClock
from concourse.bass_utils import run_bass_kernel_spmd

FP32 = mybir.dt.float32
FP32R = mybir.dt.float32r


# fp32r (single-pass PE matmul, 2x faster than fp32; ~1.6e-4 relative
# rounding, measured on HW) is threaded through tile dtypes natively: the
# BIR verifier requires every producer of an fp32r-matmul input to round.

# Problem constants (hardcoded per contract)
B, S, IN_DIM, E = 2, 2048, 512, 512
H, HD = 8, 64
WS, HW = 256, 128
CH = 512          # own queries per core
LK = 768          # local keys per core (chunk + 128 halo each side)
NT = 6            # key tiles of 128
W_T = [128, 256, 384, 384, 256, 128]   # valid query-span width per key tile
QS_T = [0, 0, 0, 128, 256, 384]        # local query start per key tile
OFF_T = [0, 128, 384, 768, 1152, 1408]  # column offset in the concat layout
WSUM = 1536

_MAX_WAITS = 1
_patched = False


def _split_sync_waits(nc):
    """This container's walrus accepts only 1 sync-wait per instruction.
    Move extra waits onto nofuse NOPs inserted just before, on the same
    engine sequencer (in-order execution makes this equivalent)."""
    n_split = 0
    for fn in nc.m.functions:
        for bb in fn.blocks:
            insts = list(bb.instructions)
            out = []
            for inst in insts:
                si = inst.sync_info
                if si is not None and len(si.on_wait) > _MAX_WAITS:
                    waits = list(si.on_wait)
                    extra, keep = waits[:-_MAX_WAITS], waits[-_MAX_WAITS:]
                    for j in range(0, len(extra), _MAX_WAITS):
                        out.append(
                            mybir.InstNoOp(
                                name=f"{inst.name}-sw{j}",
                                engine=inst.engine,
                                bass_nofuse=True,
                                sync_info=mybir.SyncInfo(
                                    on_wait=extra[j : j + _MAX_WAITS], on_update=[]
                                ),
                            )
                        )
                    inst.sync_info = mybir.SyncInfo(
                        on_wait=keep, on_update=list(si.on_update)
                    )
                    n_split += 1
                out.append(inst)
            if len(out) != len(insts):
                try:
                    bb.instructions = out
                except Exception:
                    bb.instructions[:] = out
    return n_split


def _patch_tile_drain():
    global _patched
    if _patched:
        return
    _patched = True

    def _drain_and_barrier(self, tick_clock, wait_clock):
        nc = self.nc
        drain_inst = nc.sync.drain()
        wait_clock.add_sem_waits(
            drain_inst.ins, ScopedClock({None: tick_clock.global_clock})
        )
        nc.all_engine_barrier()
        assert self.sems is not None
        popped = nc._tile_sem_poison_stack.pop()
        assert popped is self._sem_poison
        nc.clear_and_free_semaphores(list(self.sems.allocated().values()))
        nc.all_engine_barrier()
        _split_sync_waits(nc)

    tile.TileContext._drain_and_barrier = _drain_and_barrier


def _build_program():
    _patch_tile_drain()
    nc = bass.Bass("TRN2", target_bir_lowering=False, debug=False)

    xt = nc.dram_tensor("xt", [IN_DIM, LK], FP32R, kind="ExternalInput")
    wq = nc.dram_tensor("wq", [IN_DIM, E], FP32R, kind="ExternalInput")
    wk = nc.dram_tensor("wk", [IN_DIM, E], FP32R, kind="ExternalInput")
    wv = nc.dram_tensor("wv", [IN_DIM, E], FP32R, kind="ExternalInput")
    ow = nc.dram_tensor("ow", [E, E], FP32R, kind="ExternalInput")
    qb = nc.dram_tensor("qb", [4, 128], FP32, kind="ExternalInput")
    kb = nc.dram_tensor("kb", [4, 128], FP32, kind="ExternalInput")
    vb = nc.dram_tensor("vb", [128, E], FP32, kind="ExternalInput")
    ob = nc.dram_tensor("ob", [128, E], FP32, kind="ExternalInput")
    mk = nc.dram_tensor("mk", [128, WSUM], FP32R, kind="ExternalInput")
    out = nc.dram_tensor("out", [CH, E], FP32, kind="ExternalOutput")

    with tile.TileContext(nc) as tc:
        with (
            tc.tile_pool(name="const", bufs=1) as cpool,
            tc.tile_pool(name="proj", bufs=1) as ppool,
            tc.tile_pool(name="att", bufs=3) as apool,
            tc.tile_pool(name="small", bufs=2) as spool,
            tc.tile_pool(name="fin", bufs=2) as fpool,
            tc.tile_pool(name="ps2", bufs=2, space="PSUM") as ps2,
            tc.tile_pool(name="ps3", bufs=3, space="PSUM") as ps3,
            tc.tile_pool(name="ps1", bufs=1, space="PSUM") as ps1,
        ):
            # ---- loads, ordered so the q/k projections can start ASAP ----
            def load(pool_tag, shape, dt, ap):
                t = cpool.tile(shape, dt, tag=pool_tag)
                nc.sync.dma_start(t[:], ap)
                return t

            # x first, then just the weight columns the first projection
            # chunks need (pair 0), so the PE starts ~7us in instead of ~17
            xt_t = [load(f"xt{p}", [128, LK], FP32R, xt[128 * p : 128 * p + 128, :]) for p in range(4)]
            wq_t = [cpool.tile([128, E], FP32R, name=f"wq{p}", tag=f"wq{p}") for p in range(4)]
            wk_t = [cpool.tile([128, E], FP32R, name=f"wk{p}", tag=f"wk{p}") for p in range(4)]
            for kk in range(4):
                nc.sync.dma_start(wq_t[kk][:, 0:128], wq[128 * kk : 128 * kk + 128, 0:128])
            qb_t = [load(f"qb{p}", [128, 1], FP32, qb[p, :][:, None]) for p in range(4)]
            kb_t = [load(f"kb{p}", [128, 1], FP32, kb[p, :][:, None]) for p in range(4)]
            for kk in range(4):
                nc.sync.dma_start(wk_t[kk][:, 0:128], wk[128 * kk : 128 * kk + 128, 0:128])
            wv_t = [load(f"wv{p}", [128, E], FP32R, wv[128 * p : 128 * p + 128, :]) for p in range(4)]
            for kk in range(4):
                nc.sync.dma_start(wq_t[kk][:, 128:E], wq[128 * kk : 128 * kk + 128, 128:E])
            for kk in range(4):
                nc.sync.dma_start(wk_t[kk][:, 128:E], wk[128 * kk : 128 * kk + 128, 128:E])
            vb_t = load("vb", [128, E], FP32, vb[:])
            mk_t = load("mk", [128, WSUM], FP32R, mk[:])
            ow_t = [load(f"ow{p}", [128, E], FP32R, ow[128 * p : 128 * p + 128, :]) for p in range(4)]
            ob_t = load("ob", [128, E], FP32, ob[:])
            ones_t = cpool.tile([1, 64], FP32, tag="ones")
            nc.vector.memset(ones_t[:], 1.0)

            qT = [None] * 4
            kT = [None] * 4

            # q/k projection for pair p, split into 3 chunks so it can be
            # interleaved into the previous pair's attention (keeps the PE
            # dense with N=512 matmuls so HAM stays at full clock)
            def emit_qk_chunk(p, chunk):
                if chunk == 0:
                    psq = ps2.tile([128, CH], FP32, tag="ps_big")
                    for kk in range(4):
                        nc.tensor.matmul(
                            psq[:],
                            wq_t[kk][:, 128 * p : 128 * p + 128],
                            xt_t[kk][:, 128 : 128 + CH],
                            start=(kk == 0), stop=(kk == 3),
                        )
                    q = ppool.tile([128, CH], FP32R, tag=f"qT{p}")
                    nc.vector.tensor_scalar_add(q[:], psq[:], qb_t[p][:, 0:1])
                    qT[p] = q
                else:
                    h = chunk - 1
                    if h == 0:
                        kT[p] = ppool.tile([128, LK], FP32R, name=f"kT{p}", tag=f"kT{p}")
                    psk = ps3.tile([128, 384], FP32, tag="ps_s")
                    for kk in range(4):
                        nc.tensor.matmul(
                            psk[:],
                            wk_t[kk][:, 128 * p : 128 * p + 128],
                            xt_t[kk][:, 384 * h : 384 * h + 384],
                            start=(kk == 0), stop=(kk == 3),
                        )
                    nc.vector.tensor_scalar_add(
                        kT[p][:, 384 * h : 384 * h + 384], psk[:], kb_t[p][:, 0:1]
                    )

            for c in range(3):
                emit_qk_chunk(0, c)

            # v in natural layout [keys, 8*(64+1)]: per head 64 v-cols + ones
            v_t = []
            for m in range(NT):
                psv = ps2.tile([128, E], FP32, tag="ps_big")
                for kk in range(4):
                    nc.tensor.matmul(
                        psv[:],
                        xt_t[kk][:, 128 * m : 128 * m + 128],
                        wv_t[kk][:],
                        start=(kk == 0), stop=(kk == 3),
                    )
                v = ppool.tile([128, H * (HD + 1)], FP32R, tag=f"v{m}")
                v3 = v[:].rearrange("p (h d) -> p h d", d=HD + 1)
                psv3 = psv[:].rearrange("p (h d) -> p h d", d=HD)
                vb3 = vb_t[:].rearrange("p (h d) -> p h d", d=HD)
                nc.vector.tensor_tensor(v3[:, :, 0:HD], psv3, vb3, op=AluOpType.add)
                nc.vector.memset(v3[:, :, HD : HD + 1].bitcast(FP32), 1.0)
                v_t.append(v)

            # ---- attention (per pair of heads sharing a 128-row tile) ----
            # scores^T via row-packed K=64 QK pairs; the banded mask is
            # accumulated on the PE (identity @ maskneg, -50 out-of-band)
            # so exp() zeroes invalid entries with no elementwise op.
            vT = []
            for p in range(4):
                attA = apool.tile([128, WSUM], FP32R, tag="attA")
                attB = apool.tile([128, WSUM], FP32R, tag="attB")
                for t in range(NT):
                    w, qs, off = W_T[t], QS_T[t], OFF_T[t]
                    pa = ps3.tile([128, 384], FP32, tag="ps_s")
                    nc.tensor.matmul(
                        pa[:, 0:w],
                        kT[p][0:64, 128 * t : 128 * t + 128],
                        qT[p][0:64, qs : qs + w],
                        start=True, stop=True,
                    )
                    pb = ps2.tile([128, 512], FP32, tag="ps_big")
                    nc.tensor.matmul(
                        pb[:, 0:w],
                        kT[p][64:128, 128 * t : 128 * t + 128],
                        qT[p][64:128, qs : qs + w],
                        start=True, stop=True,
                    )
                    nc.scalar.activation(
                        attA[:, off : off + w], pa[:, 0:w],
                        mybir.ActivationFunctionType.Exp,
                    )
                    nc.scalar.activation(
                        attB[:, off : off + w], pb[:, 0:w],
                        mybir.ActivationFunctionType.Exp,
                    )
                    # band mask: head A on DVE, head B on GpSimd
                    nc.vector.tensor_mul(
                        attA[:, off : off + w], attA[:, off : off + w],
                        mk_t[:, off : off + w],
                    )
                    nc.gpsimd.tensor_mul(
                        attB[:, off : off + w], attB[:, off : off + w],
                        mk_t[:, off : off + w],
                    )
                    # interleave the next pair's projection matmuls to keep
                    # the PE stream dense while ACT works through the exps
                    if p < 3 and t in (1, 3, 4):
                        emit_qk_chunk(p + 1, {1: 0, 3: 1, 4: 2}[t])

                # AV: psO[0:64] = V^T @ att^T (unnormalized values^T),
                #     psO[64]   = column sums (softmax denominators).
                vtn = ppool.tile([128, CH], FP32R, tag=f"vT{p}")
                rbc = ps1.tile([128, CH], FP32, tag="rbc")
                psos = []
                for hh in range(2):
                    head = 2 * p + hh
                    att = attA if hh == 0 else attB
                    pso = ps2.tile([HD + 1, CH], FP32, tag="psO")
                    for t in range(NT):
                        w, qs, off = W_T[t], QS_T[t], OFF_T[t]
                        nc.tensor.matmul(
                            pso[:, qs : qs + w],
                            v_t[t][:, (HD + 1) * head : (HD + 1) * head + HD + 1],
                            att[:, off : off + w],
                            start=(t == 0), stop=(t == NT - 1),
                            skip_group_check=True,
                        )
                    # denominator row -> SBUF, broadcast across 64
                    # partitions with a K=1 matmul into the pair's rbc bank
                    den = spool.tile([1, CH], FP32, tag="den")
                    nc.vector.tensor_copy(den[:], pso[HD : HD + 1, :])
                    nc.tensor.matmul(
                        rbc[64 * hh : 64 * hh + 64, :], ones_t[:], den[:],
                        start=True, stop=True,
                    )
                    psos.append(pso)
                # reciprocal of both heads' denominators at once via ACT
                # exp(-ln(x)) (DVE reciprocal is ~3.4us/op; ACT Reciprocal
                # is blocked by bass), then scale values per head.
                lnv = spool.tile([128, CH], FP32, tag="lnv")
                nc.scalar.activation(
                    lnv[:], rbc[:], mybir.ActivationFunctionType.Ln,
                )
                rbs = spool.tile([128, CH], FP32, tag="rbs")
                nc.scalar.activation(
                    rbs[:], lnv[:],
                    mybir.ActivationFunctionType.Exp, scale=-1.0,
                )
                for hh in range(2):
                    nc.vector.tensor_mul(
                        vtn[64 * hh : 64 * hh + 64, :],
                        psos[hh][0:HD, :], rbs[64 * hh : 64 * hh + 64, :],
                    )
                vT.append(vtn)

            # ---- output projection ----
            for m in range(4):
                psf = ps2.tile([128, E], FP32, tag="ps_big")
                for p in range(4):
                    nc.tensor.matmul(
                        psf[:],
                        vT[p][:, 128 * m : 128 * m + 128],
                        ow_t[p][:],
                        start=(p == 0), stop=(p == 3),
                    )
                fin = fpool.tile([128, E], FP32, tag="fin")
                nc.vector.tensor_tensor(fin[:], psf[:], ob_t[:], op=AluOpType.add)
                nc.sync.dma_start(out[128 * m : 128 * m + 128, :], fin[:])

    return nc


_NC_CACHE = None


def _get_program():
    global _NC_CACHE
    if _NC_CACHE is None:
        _NC_CACHE = _build_program()
    return _NC_CACHE


def _make_in_maps(x, padding_mask, qkv_w, qkv_b, o_w, o_b):
    x = np.asarray(x, np.float32)
    pm = np.asarray(padding_mask)
    qkv_w = np.asarray(qkv_w, np.float32)
    qkv_b = np.asarray(qkv_b, np.float32)
    o_w = np.asarray(o_w, np.float32)
    o_b = np.asarray(o_b, np.float32)

    scale = np.float32(1.0 / np.sqrt(HD))
    # reference splits per-head: head h uses qkv rows [192h,192h+64) (q),
    # +64 (k), +128 (v)
    idx_q = np.concatenate([np.arange(3 * HD * h, 3 * HD * h + HD) for h in range(H)])
    idx_k = idx_q + HD
    idx_v = idx_q + 2 * HD

    wq = np.ascontiguousarray((qkv_w[idx_q] * scale).T)      # [IN, E]
    wk = np.ascontiguousarray(qkv_w[idx_k].T)
    wv = np.ascontiguousarray(qkv_w[idx_v].T)
    qb = np.ascontiguousarray((qkv_b[idx_q] * scale).reshape(4, 128))
    kb = np.ascontiguousarray(qkv_b[idx_k].reshape(4, 128))
    vb = np.ascontiguousarray(
        np.broadcast_to(qkv_b[idx_v][None, :], (128, E))
    )
    ow = np.ascontiguousarray(o_w.T)                          # [E_in, E_out]
    ob = np.ascontiguousarray(np.broadcast_to(o_b[None, :], (128, E)))

    j = np.arange(128)[:, None]
    in_maps = []
    for c in range(8):
        b, ch = divmod(c, 4)
        s0 = CH * ch
        lo, hi = max(0, s0 - HW), min(S, s0 + CH + HW)
        xpad = np.zeros((LK, IN_DIM), np.float32)
        xpad[lo - (s0 - HW) : hi - (s0 - HW)] = x[b, lo:hi]
        xt = np.ascontiguousarray(xpad.T)                     # [IN, LK]

        mask = np.zeros((128, WSUM), np.float32)
        for t in range(NT):
            w, qs, off = W_T[t], QS_T[t], OFF_T[t]
            lk = 128 * t + j                                  # [128,1] local key
            q = qs + np.arange(w)[None, :]                    # [1,w] local query
            band = (q <= lk) & (lk <= q + 2 * HW)
            gk = s0 - HW + lk                                 # global key index
            valid = (gk >= 0) & (gk < S)
            pmk = pm[b, np.clip(gk, 0, S - 1)] != 0
            mask[:, off : off + w] = (band & valid & pmk).astype(np.float32)

        in_maps.append(
            {"xt": xt, "wq": wq, "wk": wk, "wv": wv, "ow": ow,
             "qb": qb, "kb": kb, "vb": vb, "ob": ob, "mk": mask}
        )
    return in_maps


def _run(x, padding_mask, qkv_w, qkv_b, o_w, o_b, trace=False, tmpdir=None):
    nc = _get_program()
    in_maps = _make_in_maps(x, padding_mask, qkv_w, qkv_b, o_w, o_b)
    res = run_bass_kernel_spmd(
        nc, in_maps, core_ids=list(range(8)), trace=trace, tmpdir=tmpdir
    )
    o = np.empty((B, S, E), np.float32)
    for c in range(8):
        b, ch = divmod(c, 4)
        o[b, CH * ch : CH * ch + CH, :] = res.results[c]["out"]
    # fully-masked query rows: att = 0 -> output is exactly the bias
    pm = np.asarray(padding_mask)
    if (pm == 0).any():
        o[pm == 0] = np.asarray(o_b, np.float32)
    return o, res


def kernel(x, padding_mask, qkv_w, qkv_b, o_w, o_b, window_size, num_heads):
    assert int(window_size) == WS and int(num_heads) == H
    assert tuple(np.asarray(x).shape) == (B, S, IN_DIM)
    o, _ = _run(x, padding_mask, qkv_w, qkv_b, o_w, o_b)
    return o


# revision 27
# speedup vs baseline: 1.0433x; 1.0433x over previous
"""Sliding-window multi-head attention for Trainium2, 8-core SPMD.

Sharding: sequence-parallel. B=2 batches x 4 chunks of 512 queries = 8 cores.
Each core computes QKV projections for its chunk (+128-row halo for K/V),
banded attention (window 256 -> band |j-s|<=128), and the output projection
for its 512 rows. No collectives; host concatenates the 8 output chunks.

Math notes (validated against the reference):
 - The reference's clamped scatter-add with zero-padded keys is exactly a
   banded score matrix: full[s,j] = q_s.k_j / 8 for |j-s|<=128, -inf outside.
 - Softmax computed without max-subtraction (scores are O(1), no overflow).
 - Denominators come free from the AV matmul via a ones-column on V (M=65).
 - Attention is computed transposed (scores^T[key, query]) so no transposes
   are needed anywhere in the hot loop; q^T/k^T come straight out of the
   projection, V is projected in natural layout for the AV lhsT.
"""

import numpy as np

import concourse.bass as bass
import concourse.tile as tile
from concourse import mybir
from concourse.alu_op_type import AluOpType
from concourse.vector_clock import ScopedClock
from concourse.bass_utils import run_bass_kernel_spmd

FP32 = mybir.dt.float32
FP32R = mybir.dt.float32r


# fp32r (single-pass PE matmul, 2x faster than fp32; ~1.6e-4 relative
# rounding, measured on HW) is threaded through tile dtypes natively: the
# BIR verifier requires every producer of an fp32r-matmul input to round.

# Problem constants (hardcoded per contract)
B, S, IN_DIM, E = 2, 2048, 512, 512
H, HD = 8, 64
WS, HW = 256, 128
CH = 512          # own queries per core
LK = 768          # local keys per core (chunk + 128 halo each side)
NT = 6            # key tiles of 128
W_T = [128, 256, 384, 384, 256, 128]   # valid query-span width per key tile
QS_T = [0, 0, 0, 128, 256, 384]        # local query start per key tile
OFF_T = [0, 128, 384, 768, 1152, 1408]  # column offset in the concat layout
WSUM = 1536

_MAX_WAITS = 1
_patched = False


def _split_sync_waits(nc):
    """This container's walrus accepts only 1 sync-wait per instruction.
    Move extra waits onto nofuse NOPs inserted just before, on the same
    engine sequencer (in-order execution makes this equivalent)."""
    n_split = 0
    for fn in nc.m.functions:
        for bb in fn.blocks:
            insts = list(bb.instructions)
            out = []
            for inst in insts:
                si = inst.sync_info
                if si is not None and len(si.on_wait) > _MAX_WAITS:
                    waits = list(si.on_wait)
                    extra, keep = waits[:-_MAX_WAITS], waits[-_MAX_WAITS:]
                    for j in range(0, len(extra), _MAX_WAITS):
                        out.append(
                            mybir.InstNoOp(
                                name=f"{inst.name}-sw{j}",
                                engine=inst.engine,
                                bass_nofuse=True,
                                sync_info=mybir.SyncInfo(
                                    on_wait=extra[j : j + _MAX_WAITS], on_update=[]
                                ),
                            )
                        )
                    inst.sync_info = mybir.SyncInfo(
                        on_wait=keep, on_update=list(si.on_update)
                    )
                    n_split += 1
                out.append(inst)
            if len(out) != len(insts):
                try:
                    bb.instructions = out
                except Exception:
                    bb.instructions[:] = out
    return n_split


def _patch_tile_drain():
    global _patched
    if _patched:
        return
    _patched = True

    def _drain_and_barrier(self, tick_clock, wait_clock):
        nc = self.nc
        drain_inst = nc.sync.drain()
        wait_clock.add_sem_waits(
            drain_inst.ins, ScopedClock({None: tick_clock.global_clock})
        )
        nc.all_engine_barrier()
        assert self.sems is not None
        popped = nc._tile_sem_poison_stack.pop()
        assert popped is self._sem_poison
        nc.clear_and_free_semaphores(list(self.sems.allocated().values()))
        nc.all_engine_barrier()
        _split_sync_waits(nc)

    tile.TileContext._drain_and_barrier = _drain_and_barrier


def _build_program():
    _patch_tile_drain()
    nc = bass.Bass("TRN2", target_bir_lowering=False, debug=False)

    xt = nc.dram_tensor("xt", [IN_DIM, LK], FP32R, kind="ExternalInput")
    wq = nc.dram_tensor("wq", [IN_DIM, E], FP32R, kind="ExternalInput")
    wk = nc.dram_tensor("wk", [IN_DIM, E], FP32R, kind="ExternalInput")
    wv = nc.dram_tensor("wv", [IN_DIM, E], FP32R, kind="ExternalInput")
    ow = nc.dram_tensor("ow", [E, E], FP32R, kind="ExternalInput")
    qb = nc.dram_tensor("qb", [4, 128], FP32, kind="ExternalInput")
    kb = nc.dram_tensor("kb", [4, 128], FP32, kind="ExternalInput")
    vb = nc.dram_tensor("vb", [128, E], FP32, kind="ExternalInput")
    ob = nc.dram_tensor("ob", [128, E], FP32, kind="ExternalInput")
    mk = nc.dram_tensor("mk", [128, WSUM], FP32R, kind="ExternalInput")
    out = nc.dram_tensor("out", [CH, E], FP32, kind="ExternalOutput")

    with tile.TileContext(nc) as tc:
        with (
            tc.tile_pool(name="const", bufs=1) as cpool,
            tc.tile_pool(name="proj", bufs=1) as ppool,
            tc.tile_pool(name="att", bufs=3) as apool,
            tc.tile_pool(name="small", bufs=2) as spool,
            tc.tile_pool(name="fin", bufs=2) as fpool,
            tc.tile_pool(name="ps2", bufs=2, space="PSUM") as ps2,
            tc.tile_pool(name="ps3", bufs=3, space="PSUM") as ps3,
            tc.tile_pool(name="ps1", bufs=1, space="PSUM") as ps1,
        ):
            # ---- loads, ordered so the q/k projections can start ASAP ----
            def load(pool_tag, shape, dt, ap):
                t = cpool.tile(shape, dt, tag=pool_tag)
                nc.sync.dma_start(t[:], ap)
                return t

            # x first, then just the weight columns the first projection
            # chunks need (pair 0), so the PE starts ~7us in instead of ~17
            xt_t = [load(f"xt{p}", [128, LK], FP32R, xt[128 * p : 128 * p + 128, :]) for p in range(4)]
            wq_t = [cpool.tile([128, E], FP32R, name=f"wq{p}", tag=f"wq{p}") for p in range(4)]
            wk_t = [cpool.tile([128, E], FP32R, name=f"wk{p}", tag=f"wk{p}") for p in range(4)]
            for kk in range(4):
                nc.sync.dma_start(wq_t[kk][:, 0:128], wq[128 * kk : 128 * kk + 128, 0:128])
            qb_t = [load(f"qb{p}", [128, 1], FP32, qb[p, :][:, None]) for p in range(4)]
            kb_t = [load(f"kb{p}", [128, 1], FP32, kb[p, :][:, None]) for p in range(4)]
            for kk in range(4):
                nc.sync.dma_start(wk_t[kk][:, 0:128], wk[128 * kk : 128 * kk + 128, 0:128])
            wv_t = [load(f"wv{p}", [128, E], FP32R, wv[128 * p : 128 * p + 128, :]) for p in range(4)]
            for kk in range(4):
                nc.sync.dma_start(wq_t[kk][:, 128:E], wq[128 * kk : 128 * kk + 128, 128:E])
            for kk in range(4):
                nc.sync.dma_start(wk_t[kk][:, 128:E], wk[128 * kk : 128 * kk + 128, 128:E])
            vb_t = load("vb", [128, E], FP32, vb[:])
            mk_t = load("mk", [128, WSUM], FP32R, mk[:])
            ow_t = [load(f"ow{p}", [128, E], FP32R, ow[128 * p : 128 * p + 128, :]) for p in range(4)]
            ob_t = load("ob", [128, E], FP32, ob[:])
            ones_t = cpool.tile([1, 64], FP32, tag="ones")
            nc.vector.memset(ones_t[:], 1.0)

            qT = [None] * 4
            kT = [None] * 4

            # q/k projection for pair p, split into 3 chunks so it can be
            # interleaved into the previous pair's attention (keeps the PE
            # dense with N=512 matmuls so HAM stays at full clock)
            def emit_qk_chunk(p, chunk):
                if chunk == 0:
                    psq = ps2.tile([128, CH], FP32, tag="ps_big")
                    for kk in range(4):
                        nc.tensor.matmul(
                            psq[:],
                            wq_t[kk][:, 128 * p : 128 * p + 128],
                            xt_t[kk][:, 128 : 128 + CH],
                            start=(kk == 0), stop=(kk == 3),
                        )
                    q = ppool.tile([128, CH], FP32R, tag=f"qT{p}")
                    nc.vector.tensor_scalar_add(q[:], psq[:], qb_t[p][:, 0:1])
                    qT[p] = q
                else:
                    h = chunk - 1
                    if h == 0:
                        kT[p] = ppool.tile([128, LK], FP32R, name=f"kT{p}", tag=f"kT{p}")
                    psk = ps3.tile([128, 384], FP32, tag="ps_s")
                    for kk in range(4):
                        nc.tensor.matmul(
                            psk[:],
                            wk_t[kk][:, 128 * p : 128 * p + 128],
                            xt_t[kk][:, 384 * h : 384 * h + 384],
                            start=(kk == 0), stop=(kk == 3),
                        )
                    nc.vector.tensor_scalar_add(
                        kT[p][:, 384 * h : 384 * h + 384], psk[:], kb_t[p][:, 0:1]
                    )

            for p in range(4):
                for c in range(3):
                    emit_qk_chunk(p, c)

            # v in natural layout [keys, 8*(64+1)]: per head 64 v-cols + ones
            v_t = []
            for m in range(NT):
                psv = ps2.tile([128, E], FP32, tag="ps_big")
                for kk in range(4):
                    nc.tensor.matmul(
                        psv[:],
                        xt_t[kk][:, 128 * m : 128 * m + 128],
                        wv_t[kk][:],
                        start=(kk == 0), stop=(kk == 3),
                    )
                v = ppool.tile([128, H * (HD + 1)], FP32R, tag=f"v{m}")
                v3 = v[:].rearrange("p (h d) -> p h d", d=HD + 1)
                psv3 = psv[:].rearrange("p (h d) -> p h d", d=HD)
                vb3 = vb_t[:].rearrange("p (h d) -> p h d", d=HD)
                nc.vector.tensor_tensor(v3[:, :, 0:HD], psv3, vb3, op=AluOpType.add)
                nc.vector.memset(v3[:, :, HD : HD + 1].bitcast(FP32), 1.0)
                v_t.append(v)

            # ---- attention (per pair of heads sharing a 128-row tile) ----
            # scores^T via row-packed K=64 QK pairs; the banded mask is
            # accumulated on the PE (identity @ maskneg, -50 out-of-band)
            # so exp() zeroes invalid entries with no elementwise op.
            vT = []
            for p in range(4):
                attA = apool.tile([128, WSUM], FP32R, tag="attA")
                attB = apool.tile([128, WSUM], FP32R, tag="attB")
                for t in range(NT):
                    w, qs, off = W_T[t], QS_T[t], OFF_T[t]
                    pa = ps3.tile([128, 384], FP32, tag="ps_s")
                    nc.tensor.matmul(
                        pa[:, 0:w],
                        kT[p][0:64, 128 * t : 128 * t + 128],
                        qT[p][0:64, qs : qs + w],
                        start=True, stop=True,
                    )
                    pb = ps2.tile([128, 512], FP32, tag="ps_big")
                    nc.tensor.matmul(
                        pb[:, 0:w],
                        kT[p][64:128, 128 * t : 128 * t + 128],
                        qT[p][64:128, qs : qs + w],
                        start=True, stop=True,
                    )
                    nc.scalar.activation(
                        attA[:, off : off + w], pa[:, 0:w],
                        mybir.ActivationFunctionType.Exp,
                    )
                    nc.scalar.activation(
                        attB[:, off : off + w], pb[:, 0:w],
                        mybir.ActivationFunctionType.Exp,
                    )
                    # band mask: head A on DVE, head B on GpSimd
                    nc.vector.tensor_mul(
                        attA[:, off : off + w], attA[:, off : off + w],
                        mk_t[:, off : off + w],
                    )
                    nc.gpsimd.tensor_mul(
                        attB[:, off : off + w], attB[:, off : off + w],
                        mk_t[:, off : off + w],
                    )
                # AV: psO[0:64] = V^T @ att^T (unnormalized values^T),
                #     psO[64]   = column sums (softmax denominators).
                vtn = ppool.tile([128, CH], FP32R, tag=f"vT{p}")
                rbc = ps1.tile([128, CH], FP32, tag="rbc")
                psos = []
                for hh in range(2):
                    head = 2 * p + hh
                    att = attA if hh == 0 else attB
                    pso = ps2.tile([HD + 1, CH], FP32, tag="psO")
                    for t in range(NT):
                        w, qs, off = W_T[t], QS_T[t], OFF_T[t]
                        nc.tensor.matmul(
                            pso[:, qs : qs + w],
                            v_t[t][:, (HD + 1) * head : (HD + 1) * head + HD + 1],
                            att[:, off : off + w],
                            start=(t == 0), stop=(t == NT - 1),
                            skip_group_check=True,
                        )
                    # denominator row -> SBUF, broadcast across 64
                    # partitions with a K=1 matmul into the pair's rbc bank
                    den = spool.tile([1, CH], FP32, tag="den")
                    nc.vector.tensor_copy(den[:], pso[HD : HD + 1, :])
                    nc.tensor.matmul(
                        rbc[64 * hh : 64 * hh + 64, :], ones_t[:], den[:],
                        start=True, stop=True,
                    )
                    psos.append(pso)
                # reciprocal of both heads' denominators at once via ACT
                # exp(-ln(x)) (DVE reciprocal is ~3.4us/op; ACT Reciprocal
                # is blocked by bass), then scale values per head.
                lnv = spool.tile([128, CH], FP32, tag="lnv")
                nc.scalar.activation(
                    lnv[:], rbc[:], mybir.ActivationFunctionType.Ln,
                )
                rbs = spool.tile([128, CH], FP32, tag="rbs")
                nc.scalar.activation(
                    rbs[:], lnv[:],
                    mybir.ActivationFunctionType.Exp, scale=-1.0,
                )
                for hh in range(2):
                    nc.vector.tensor_mul(
                        vtn[64 * hh : 64 * hh + 64, :],
                        psos[hh][0:HD, :], rbs[64 * hh : 64 * hh + 64, :],
                    )
                vT.append(vtn)

            # ---- output projection ----
            for m in range(4):
                psf = ps2.tile([128, E], FP32, tag="ps_big")
                for p in range(4):
                    nc.tensor.matmul(
                        psf[:],
                        vT[p][:, 128 * m : 128 * m + 128],
                        ow_t[p][:],
                        start=(p == 0), stop=(p == 3),
                    )
                fin = fpool.tile([128, E], FP32, tag="fin")
                nc.vector.tensor_tensor(fin[:], psf[:], ob_t[:], op=AluOpType.add)
                nc.sync.dma_start(out[128 * m : 128 * m + 128, :], fin[:])

    return nc


_NC_CACHE = None


def _get_program():
    global _NC_CACHE
    if _NC_CACHE is None:
        _NC_CACHE = _build_program()
    return _NC_CACHE


def _make_in_maps(x, padding_mask, qkv_w, qkv_b, o_w, o_b):
    x = np.asarray(x, np.float32)
    pm = np.asarray(padding_mask)
    qkv_w = np.asarray(qkv_w, np.float32)
    qkv_b = np.asarray(qkv_b, np.float32)
    o_w = np.asarray(o_w, np.float32)
    o_b = np.asarray(o_b, np.float32)

    scale = np.float32(1.0 / np.sqrt(HD))
    # reference splits per-head: head h uses qkv rows [192h,192h+64) (q),
    # +64 (k), +128 (v)
    idx_q = np.concatenate([np.arange(3 * HD * h, 3 * HD * h + HD) for h in range(H)])
    idx_k = idx_q + HD
    idx_v = idx_q + 2 * HD

    wq = np.ascontiguousarray((qkv_w[idx_q] * scale).T)      # [IN, E]
    wk = np.ascontiguousarray(qkv_w[idx_k].T)
    wv = np.ascontiguousarray(qkv_w[idx_v].T)
    qb = np.ascontiguousarray((qkv_b[idx_q] * scale).reshape(4, 128))
    kb = np.ascontiguousarray(qkv_b[idx_k].reshape(4, 128))
    vb = np.ascontiguousarray(
        np.broadcast_to(qkv_b[idx_v][None, :], (128, E))
    )
    ow = np.ascontiguousarray(o_w.T)                          # [E_in, E_out]
    ob = np.ascontiguousarray(np.broadcast_to(o_b[None, :], (128, E)))

    j = np.arange(128)[:, None]
    in_maps = []
    for c in range(8):
        b, ch = divmod(c, 4)
        s0 = CH * ch
        lo, hi = max(0, s0 - HW), min(S, s0 + CH + HW)
        xpad = np.zeros((LK, IN_DIM), np.float32)
        xpad[lo - (s0 - HW) : hi - (s0 - HW)] = x[b, lo:hi]
        xt = np.ascontiguousarray(xpad.T)                     # [IN, LK]

        mask = np.zeros((128, WSUM), np.float32)
        for t in range(NT):
            w, qs, off = W_T[t], QS_T[t], OFF_T[t]
            lk = 128 * t + j                                  # [128,1] local key
            q = qs + np.arange(w)[None, :]                    # [1,w] local query
            band = (q <= lk) & (lk <= q + 2 * HW)
            gk = s0 - HW + lk                                 # global key index
            valid = (gk >= 0) & (gk < S)
            pmk = pm[b, np.clip(gk, 0, S - 1)] != 0
            mask[:, off : off + w] = (band & valid & pmk).astype(np.float32)

        in_maps.append(
            {"xt": xt, "wq": wq, "wk": wk, "wv": wv, "ow": ow,
             "qb": qb, "kb": kb, "vb": vb, "ob": ob, "mk": mask}
        )
    return in_maps


def _run(x, padding_mask, qkv_w, qkv_b, o_w, o_b, trace=False, tmpdir=None):
    nc = _get_program()
    in_maps = _make_in_maps(x, padding_mask, qkv_w, qkv_b, o_w, o_b)
    res = run_bass_kernel_spmd(
        nc, in_maps, core_ids=list(range(8)), trace=trace, tmpdir=tmpdir
    )
    o = np.empty((B, S, E), np.float32)
    for c in range(8):
        b, ch = divmod(c, 4)
        o[b, CH * ch : CH * ch + CH, :] = res.results[c]["out"]
    # fully-masked query rows: att = 0 -> output is exactly the bias
    pm = np.asarray(padding_mask)
    if (pm == 0).any():
        o[pm == 0] = np.asarray(o_b, np.float32)
    return o, res


def kernel(x, padding_mask, qkv_w, qkv_b, o_w, o_b, window_size, num_heads):
    assert int(window_size) == WS and int(num_heads) == H
    assert tuple(np.asarray(x).shape) == (B, S, IN_DIM)
    o, _ = _run(x, padding_mask, qkv_w, qkv_b, o_w, o_b)
    return o


# revision 29
# speedup vs baseline: 1.0879x; 1.0427x over previous
"""Sliding-window multi-head attention for Trainium2, 8-core SPMD.

Sharding: sequence-parallel. B=2 batches x 4 chunks of 512 queries = 8 cores.
Each core computes QKV projections for its chunk (+128-row halo for K/V),
banded attention (window 256 -> band |j-s|<=128), and the output projection
for its 512 rows. No collectives; host concatenates the 8 output chunks.

Math notes (validated against the reference):
 - The reference's clamped scatter-add with zero-padded keys is exactly a
   banded score matrix: full[s,j] = q_s.k_j / 8 for |j-s|<=128, -inf outside.
 - Softmax computed without max-subtraction (scores are O(1), no overflow).
 - Denominators come free from the AV matmul via a ones-column on V (M=65).
 - Attention is computed transposed (scores^T[key, query]) so no transposes
   are needed anywhere in the hot loop; q^T/k^T come straight out of the
   projection, V is projected in natural layout for the AV lhsT.
"""

import numpy as np

import concourse.bass as bass
import concourse.tile as tile
from concourse import mybir
from concourse.alu_op_type import AluOpType
from concourse.vector_clock import ScopedClock
from concourse.bass_utils import run_bass_kernel_spmd

FP32 = mybir.dt.float32
FP32R = mybir.dt.float32r


# fp32r (single-pass PE matmul, 2x faster than fp32; ~1.6e-4 relative
# rounding, measured on HW) is threaded through tile dtypes natively: the
# BIR verifier requires every producer of an fp32r-matmul input to round.

# Problem constants (hardcoded per contract)
B, S, IN_DIM, E = 2, 2048, 512, 512
H, HD = 8, 64
WS, HW = 256, 128
CH = 512          # own queries per core
LK = 768          # local keys per core (chunk + 128 halo each side)
NT = 6            # key tiles of 128
W_T = [128, 256, 384, 384, 256, 128]   # valid query-span width per key tile
QS_T = [0, 0, 0, 128, 256, 384]        # local query start per key tile
OFF_T = [0, 128, 384, 768, 1152, 1408]  # column offset in the concat layout
WSUM = 1536

_MAX_WAITS = 1
_patched = False


def _split_sync_waits(nc):
    """This container's walrus accepts only 1 sync-wait per instruction.
    Move extra waits onto nofuse NOPs inserted just before, on the same
    engine sequencer (in-order execution makes this equivalent)."""
    n_split = 0
    for fn in nc.m.functions:
        for bb in fn.blocks:
            insts = list(bb.instructions)
            out = []
            for inst in insts:
                si = inst.sync_info
                if si is not None and len(si.on_wait) > _MAX_WAITS:
                    waits = list(si.on_wait)
                    extra, keep = waits[:-_MAX_WAITS], waits[-_MAX_WAITS:]
                    for j in range(0, len(extra), _MAX_WAITS):
                        out.append(
                            mybir.InstNoOp(
                                name=f"{inst.name}-sw{j}",
                                engine=inst.engine,
                                bass_nofuse=True,
                                sync_info=mybir.SyncInfo(
                                    on_wait=extra[j : j + _MAX_WAITS], on_update=[]
                                ),
                            )
                        )
                    inst.sync_info = mybir.SyncInfo(
                        on_wait=keep, on_update=list(si.on_update)
                    )
                    n_split += 1
                out.append(inst)
            if len(out) != len(insts):
                try:
                    bb.instructions = out
                except Exception:
                    bb.instructions[:] = out
    return n_split


def _patch_tile_drain():
    global _patched
    if _patched:
        return
    _patched = True

    def _drain_and_barrier(self, tick_clock, wait_clock):
        nc = self.nc
        drain_inst = nc.sync.drain()
        wait_clock.add_sem_waits(
            drain_inst.ins, ScopedClock({None: tick_clock.global_clock})
        )
        nc.all_engine_barrier()
        assert self.sems is not None
        popped = nc._tile_sem_poison_stack.pop()
        assert popped is self._sem_poison
        nc.clear_and_free_semaphores(list(self.sems.allocated().values()))
        nc.all_engine_barrier()
        _split_sync_waits(nc)

    tile.TileContext._drain_and_barrier = _drain_and_barrier


def _build_program():
    _patch_tile_drain()
    nc = bass.Bass("TRN2", target_bir_lowering=False, debug=False)

    xt = nc.dram_tensor("xt", [IN_DIM, LK], FP32R, kind="ExternalInput")
    wq = nc.dram_tensor("wq", [IN_DIM, E], FP32R, kind="ExternalInput")
    wk = nc.dram_tensor("wk", [IN_DIM, E], FP32R, kind="ExternalInput")
    wv = nc.dram_tensor("wv", [IN_DIM, E], FP32R, kind="ExternalInput")
    ow = nc.dram_tensor("ow", [E, E], FP32R, kind="ExternalInput")
    qb = nc.dram_tensor("qb", [4, 128], FP32, kind="ExternalInput")
    kb = nc.dram_tensor("kb", [4, 128], FP32, kind="ExternalInput")
    vb = nc.dram_tensor("vb", [128, E], FP32, kind="ExternalInput")
    ob = nc.dram_tensor("ob", [128, E], FP32, kind="ExternalInput")
    mk = nc.dram_tensor("mk", [128, WSUM], FP32R, kind="ExternalInput")
    out = nc.dram_tensor("out", [CH, E], FP32, kind="ExternalOutput")

    with tile.TileContext(nc) as tc:
        with (
            tc.tile_pool(name="const", bufs=1) as cpool,
            tc.tile_pool(name="proj", bufs=1) as ppool,
            tc.tile_pool(name="att", bufs=3) as apool,
            tc.tile_pool(name="small", bufs=2) as spool,
            tc.tile_pool(name="fin", bufs=2) as fpool,
            tc.tile_pool(name="ps2", bufs=2, space="PSUM") as ps2,
            tc.tile_pool(name="ps3", bufs=3, space="PSUM") as ps3,
            tc.tile_pool(name="ps1", bufs=1, space="PSUM") as ps1,
        ):
            # ---- loads, ordered so the q/k projections can start ASAP ----
            def load(pool_tag, shape, dt, ap):
                t = cpool.tile(shape, dt, tag=pool_tag)
                nc.sync.dma_start(t[:], ap)
                return t

            # x + weights; alternate the two HWDGE queues (sync=SP,
            # scalar=ACT) so the ~0.6us per-DMA dispatch doesn't serialize
            # the front of the kernel.
            def load(pool_tag, shape, dt, ap, eng):
                t = cpool.tile(shape, dt, tag=pool_tag, name=pool_tag)
                eng.dma_start(t[:], ap)
                return t

            xt_t = [load(f"xt{p}", [128, LK], FP32R, xt[128 * p : 128 * p + 128, :],
                         nc.sync if p % 2 == 0 else nc.scalar) for p in range(4)]
            wq_t = [load(f"wq{p}", [128, E], FP32R, wq[128 * p : 128 * p + 128, :],
                         nc.sync if p % 2 == 0 else nc.scalar) for p in range(4)]
            qb_t = [load(f"qb{p}", [128, 1], FP32, qb[p, :][:, None], nc.sync) for p in range(4)]
            kb_t = [load(f"kb{p}", [128, 1], FP32, kb[p, :][:, None], nc.scalar) for p in range(4)]
            wk_t = [load(f"wk{p}", [128, E], FP32R, wk[128 * p : 128 * p + 128, :],
                         nc.sync if p % 2 == 0 else nc.scalar) for p in range(4)]
            wv_t = [load(f"wv{p}", [128, E], FP32R, wv[128 * p : 128 * p + 128, :],
                         nc.sync if p % 2 == 0 else nc.scalar) for p in range(4)]
            vb_t = load("vb", [128, E], FP32, vb[:], nc.scalar)
            mk_t = load("mk", [128, WSUM], FP32R, mk[:], nc.sync)
            ow_t = [load(f"ow{p}", [128, E], FP32R, ow[128 * p : 128 * p + 128, :],
                         nc.sync if p % 2 == 0 else nc.scalar) for p in range(4)]
            ob_t = load("ob", [128, E], FP32, ob[:], nc.scalar)
            ones_t = cpool.tile([1, 64], FP32, tag="ones")
            nc.vector.memset(ones_t[:], 1.0)

            qT = [None] * 4
            kT = [None] * 4

            # q/k projection for pair p, split into 3 chunks so it can be
            # interleaved into the previous pair's attention (keeps the PE
            # dense with N=512 matmuls so HAM stays at full clock)
            def emit_qk_chunk(p, chunk):
                if chunk == 0:
                    psq = ps2.tile([128, CH], FP32, tag="ps_big")
                    for kk in range(4):
                        nc.tensor.matmul(
                            psq[:],
                            wq_t[kk][:, 128 * p : 128 * p + 128],
                            xt_t[kk][:, 128 : 128 + CH],
                            start=(kk == 0), stop=(kk == 3),
                        )
                    q = ppool.tile([128, CH], FP32R, tag=f"qT{p}")
                    nc.vector.tensor_scalar_add(q[:], psq[:], qb_t[p][:, 0:1])
                    qT[p] = q
                else:
                    h = chunk - 1
                    if h == 0:
                        kT[p] = ppool.tile([128, LK], FP32R, name=f"kT{p}", tag=f"kT{p}")
                    psk = ps3.tile([128, 384], FP32, tag="ps_s")
                    for kk in range(4):
                        nc.tensor.matmul(
                            psk[:],
                            wk_t[kk][:, 128 * p : 128 * p + 128],
                            xt_t[kk][:, 384 * h : 384 * h + 384],
                            start=(kk == 0), stop=(kk == 3),
                        )
                    nc.vector.tensor_scalar_add(
                        kT[p][:, 384 * h : 384 * h + 384], psk[:], kb_t[p][:, 0:1]
                    )

            for p in range(4):
                for c in range(3):
                    emit_qk_chunk(p, c)

            # v in natural layout [keys, 8*(64+1)]: per head 64 v-cols + ones
            v_t = []
            for m in range(NT):
                psv = ps2.tile([128, E], FP32, tag="ps_big")
                for kk in range(4):
                    nc.tensor.matmul(
                        psv[:],
                        xt_t[kk][:, 128 * m : 128 * m + 128],
                        wv_t[kk][:],
                        start=(kk == 0), stop=(kk == 3),
                    )
                v = ppool.tile([128, H * (HD + 1)], FP32R, tag=f"v{m}")
                v3 = v[:].rearrange("p (h d) -> p h d", d=HD + 1)
                psv3 = psv[:].rearrange("p (h d) -> p h d", d=HD)
                vb3 = vb_t[:].rearrange("p (h d) -> p h d", d=HD)
                nc.vector.tensor_tensor(v3[:, :, 0:HD], psv3, vb3, op=AluOpType.add)
                nc.vector.memset(v3[:, :, HD : HD + 1].bitcast(FP32), 1.0)
                v_t.append(v)

            # ---- attention (per pair of heads sharing a 128-row tile) ----
            # scores^T via row-packed K=64 QK pairs, exp on ACT, band mask
            # split across DVE (head A) / GpSimd (head B). Emission is
            # pipeline-shifted: QK phase of pair p+1 goes BEFORE the AV
            # phase of pair p, so the PE has independent matmuls to run
            # while ACT/DVE/GpSimd chew through pair p's exps and masks.
            att_tiles = {}

            def emit_qk_phase(p):
                attA = apool.tile([128, WSUM], FP32R, name=f"attA{p}", tag="attA")
                attB = apool.tile([128, WSUM], FP32R, name=f"attB{p}", tag="attB")
                att_tiles[p] = (attA, attB)
                for t in range(NT):
                    w, qs, off = W_T[t], QS_T[t], OFF_T[t]
                    pa = ps3.tile([128, 384], FP32, name=f"pa{p}_{t}", tag="ps_s")
                    nc.tensor.matmul(
                        pa[:, 0:w],
                        kT[p][0:64, 128 * t : 128 * t + 128],
                        qT[p][0:64, qs : qs + w],
                        start=True, stop=True,
                    )
                    pb = ps2.tile([128, 512], FP32, name=f"pb{p}_{t}", tag="ps_big")
                    nc.tensor.matmul(
                        pb[:, 0:w],
                        kT[p][64:128, 128 * t : 128 * t + 128],
                        qT[p][64:128, qs : qs + w],
                        start=True, stop=True,
                    )
                    nc.scalar.activation(
                        attA[:, off : off + w], pa[:, 0:w],
                        mybir.ActivationFunctionType.Exp,
                    )
                    nc.scalar.activation(
                        attB[:, off : off + w], pb[:, 0:w],
                        mybir.ActivationFunctionType.Exp,
                    )
                    nc.vector.tensor_mul(
                        attA[:, off : off + w], attA[:, off : off + w],
                        mk_t[:, off : off + w],
                    )
                    nc.gpsimd.tensor_mul(
                        attB[:, off : off + w], attB[:, off : off + w],
                        mk_t[:, off : off + w],
                    )

            def emit_av_phase(p):
                # AV: psO[0:64] = V^T @ att^T (unnormalized values^T),
                #     psO[64]   = column sums (softmax denominators).
                attA, attB = att_tiles[p]
                vtn = ppool.tile([128, CH], FP32R, name=f"vT{p}", tag=f"vT{p}")
                rbc = ps1.tile([128, CH], FP32, name=f"rbc{p}", tag="rbc")
                psos = []
                for hh in range(2):
                    head = 2 * p + hh
                    att = attA if hh == 0 else attB
                    pso = ps2.tile([HD + 1, CH], FP32, name=f"pso{head}", tag="psO")
                    for t in range(NT):
                        w, qs, off = W_T[t], QS_T[t], OFF_T[t]
                        nc.tensor.matmul(
                            pso[:, qs : qs + w],
                            v_t[t][:, (HD + 1) * head : (HD + 1) * head + HD + 1],
                            att[:, off : off + w],
                            start=(t == 0), stop=(t == NT - 1),
                            skip_group_check=True,
                        )
                    # denominator row -> SBUF, broadcast across 64
                    # partitions with a K=1 matmul into the pair's rbc bank
                    den = spool.tile([1, CH], FP32, name=f"den{head}", tag="den")
                    nc.vector.tensor_copy(den[:], pso[HD : HD + 1, :])
                    nc.tensor.matmul(
                        rbc[64 * hh : 64 * hh + 64, :], ones_t[:], den[:],
                        start=True, stop=True,
                    )
                    psos.append(pso)
                # reciprocal of both heads' denominators at once via ACT
                # exp(-ln(x)) (DVE reciprocal is ~3.4us/op; ACT Reciprocal
                # is blocked by bass), then scale values per head.
                lnv = spool.tile([128, CH], FP32, name=f"lnv{p}", tag="lnv")
                nc.scalar.activation(
                    lnv[:], rbc[:], mybir.ActivationFunctionType.Ln,
                )
                rbs = spool.tile([128, CH], FP32, name=f"rbs{p}", tag="rbs")
                nc.scalar.activation(
                    rbs[:], lnv[:],
                    mybir.ActivationFunctionType.Exp, scale=-1.0,
                )
                for hh in range(2):
                    nc.vector.tensor_mul(
                        vtn[64 * hh : 64 * hh + 64, :],
                        psos[hh][0:HD, :], rbs[64 * hh : 64 * hh + 64, :],
                    )
                vT.append(vtn)

            vT = []
            emit_qk_phase(0)
            for p in range(4):
                if p < 3:
                    emit_qk_phase(p + 1)
                emit_av_phase(p)

            # ---- output projection ----
            for m in range(4):
                psf = ps2.tile([128, E], FP32, tag="ps_big")
                for p in range(4):
                    nc.tensor.matmul(
                        psf[:],
                        vT[p][:, 128 * m : 128 * m + 128],
                        ow_t[p][:],
                        start=(p == 0), stop=(p == 3),
                    )
                fin = fpool.tile([128, E], FP32, tag="fin")
                nc.vector.tensor_tensor(fin[:], psf[:], ob_t[:], op=AluOpType.add)
                nc.sync.dma_start(out[128 * m : 128 * m + 128, :], fin[:])

    return nc


_NC_CACHE = None


def _get_program():
    global _NC_CACHE
    if _NC_CACHE is None:
        _NC_CACHE = _build_program()
    return _NC_CACHE


def _make_in_maps(x, padding_mask, qkv_w, qkv_b, o_w, o_b):
    x = np.asarray(x, np.float32)
    pm = np.asarray(padding_mask)
    qkv_w = np.asarray(qkv_w, np.float32)
    qkv_b = np.asarray(qkv_b, np.float32)
    o_w = np.asarray(o_w, np.float32)
    o_b = np.asarray(o_b, np.float32)

    scale = np.float32(1.0 / np.sqrt(HD))
    # reference splits per-head: head h uses qkv rows [192h,192h+64) (q),
    # +64 (k), +128 (v)
    idx_q = np.concatenate([np.arange(3 * HD * h, 3 * HD * h + HD) for h in range(H)])
    idx_k = idx_q + HD
    idx_v = idx_q + 2 * HD

    wq = np.ascontiguousarray((qkv_w[idx_q] * scale).T)      # [IN, E]
    wk = np.ascontiguousarray(qkv_w[idx_k].T)
    wv = np.ascontiguousarray(qkv_w[idx_v].T)
    qb = np.ascontiguousarray((qkv_b[idx_q] * scale).reshape(4, 128))
    kb = np.ascontiguousarray(qkv_b[idx_k].reshape(4, 128))
    vb = np.ascontiguousarray(
        np.broadcast_to(qkv_b[idx_v][None, :], (128, E))
    )
    ow = np.ascontiguousarray(o_w.T)                          # [E_in, E_out]
    ob = np.ascontiguousarray(np.broadcast_to(o_b[None, :], (128, E)))

    j = np.arange(128)[:, None]
    in_maps = []
    for c in range(8):
        b, ch = divmod(c, 4)
        s0 = CH * ch
        lo, hi = max(0, s0 - HW), min(S, s0 + CH + HW)
        xpad = np.zeros((LK, IN_DIM), np.float32)
        xpad[lo - (s0 - HW) : hi - (s0 - HW)] = x[b, lo:hi]
        xt = np.ascontiguousarray(xpad.T)                     # [IN, LK]

        mask = np.zeros((128, WSUM), np.float32)
        for t in range(NT):
            w, qs, off = W_T[t], QS_T[t], OFF_T[t]
            lk = 128 * t + j                                  # [128,1] local key
            q = qs + np.arange(w)[None, :]                    # [1,w] local query
            band = (q <= lk) & (lk <= q + 2 * HW)
            gk = s0 - HW + lk                                 # global key index
            valid = (gk >= 0) & (gk < S)
            pmk = pm[b, np.clip(gk, 0, S - 1)] != 0
            mask[:, off : off + w] = (band & valid & pmk).astype(np.float32)

        in_maps.append(
            {"xt": xt, "wq": wq, "wk": wk, "wv": wv, "ow": ow,
             "qb": qb, "kb": kb, "vb": vb, "ob": ob, "mk": mask}
        )
    return in_maps


def _run(x, padding_mask, qkv_w, qkv_b, o_w, o_b, trace=False, tmpdir=None):
    nc = _get_program()
    in_maps = _make_in_maps(x, padding_mask, qkv_w, qkv_b, o_w, o_b)
    res = run_bass_kernel_spmd(
        nc, in_maps, core_ids=list(range(8)), trace=trace, tmpdir=tmpdir
    )
    o = np.empty((B, S, E), np.float32)
    for c in range(8):
        b, ch = divmod(c, 4)
        o[b, CH * ch : CH * ch + CH, :] = res.results[c]["out"]
    # fully-masked query rows: att = 0 -> output is exactly the bias
    pm = np.asarray(padding_mask)
    if (pm == 0).any():
        o[pm == 0] = np.asarray(o_b, np.float32)
    return o, res


def kernel(x, padding_mask, qkv_w, qkv_b, o_w, o_b, window_size, num_heads):
    assert int(window_size) == WS and int(num_heads) == H
    assert tuple(np.asarray(x).shape) == (B, S, IN_DIM)
    o, _ = _run(x, padding_mask, qkv_w, qkv_b, o_w, o_b)
    return o
